# revision 26
# baseline (speedup 1.0000x reference)
"""CLUB loss kernel for Trainium2, 8-core data-parallel SPMD.

Math: with flat_x (N,D) [from x (B,D,H,W) -> (B*H*W, D)], v = exp(-p_logvar),
  loss = mean_i[ -0.5*sum_d ((x-mu)^2 - (m2 - 2*mu*m1 + mu^2)) * v ]
       = (-0.5/N) * [ A - 2B - dot(m2, V) + 2*dot(m1, W) ]
where
  A  = sum_{i,d} x^2 v          B  = sum_{i,d} x mu v
  V_d = sum_i v                 W_d = sum_i mu v
  m1 = S1/N, m2 = S2/N,  S1_d = sum_i x,  S2_d = sum_i x^2
All terms are per-core-local partial sums; the tiny (~KB) cross-core
reduction and final dot products happen on host in float64. No collectives.

Per-core device work (shard = 2048 rows = 2 b-blocks):
  - load x native (d-major), transpose 128x128 blocks on PE into PSUM
  - ACT: v = exp(-lv), p = square(xT), S1 via Copy+accum on native x
  - DVE: w = v*mu, fused scalar_tensor_tensor (mul+row-reduce) for A and B
  - PE: ones-matmul column sums for V, W, S2 in three PE column groups
    (separate PSUM banks, output partitions 0/32/64) accumulated over tiles
"""

import sys

import numpy as np

for _p in ("/opt/trn_rl_repo",):
    if _p not in sys.path:
        sys.path.append(_p)

B, D, H, W = 16, 512, 32, 32
HW = H * W
N = B * HW
NCORES = 8
BLKB = B // NCORES          # b-blocks per core (2)
ROWS = N // NCORES          # rows per core (2048)
NT = ROWS // 128            # 128-row tiles per core (16)
NDC = D // 128              # d chunks (4)
SLAB = 4                    # i-tiles per mu/lv DMA slab

_prog_cache = {}


def build_program():
    import concourse.bacc as bacc
    import concourse.tile as tile
    from concourse import mybir

    f32 = mybir.dt.float32
    AF = mybir.ActivationFunctionType
    OP = mybir.AluOpType

    nc = bacc.Bacc(
        "TRN2",
        target_bir_lowering=False,
        debug=False,
        enable_asserts=False,
        num_devices=NCORES,
    )

    x_d = nc.dram_tensor("x_s", (BLKB, D, HW), f32, kind="ExternalInput").ap()
    mu_d = nc.dram_tensor("mu_s", (ROWS, D), f32, kind="ExternalInput").ap()
    lv_d = nc.dram_tensor("lv_s", (ROWS, D), f32, kind="ExternalInput").ap()
    id_d = nc.dram_tensor("ident", (128, 128), f32, kind="ExternalInput").ap()

    # o_vws rows: 0 -> V, 1 -> W, 2 -> S2
    o_vws = nc.dram_tensor("o_vws", (3, D), f32, kind="ExternalOutput").ap()
    # o_misc cols: 0:NT -> rA, NT:2NT -> rB, 2NT:2NT+8 -> s1c
    o_misc = nc.dram_tensor(
        "o_misc", (128, 2 * NT + BLKB * NDC), f32, kind="ExternalOutput"
    ).ap()

    with tile.TileContext(nc) as tc:
        with (
            tc.tile_pool(name="const", bufs=1) as constp,
            tc.tile_pool(name="xnat", bufs=1) as xp,
            tc.tile_pool(name="slab", bufs=3) as slp,
            tc.tile_pool(name="stream", bufs=4) as sp,
            tc.tile_pool(name="accum", bufs=1) as accp,
            tc.tile_pool(name="scr", bufs=2) as scrp,
            tc.tile_pool(name="psum", bufs=2, space="PSUM") as pp,
            tc.tile_pool(name="psacc", bufs=1, space="PSUM") as pacc,
        ):
            ident = constp.tile([128, 128], f32)
            nc.sync.dma_start(ident[:], id_d[:])
            ones = constp.tile([128, 1], f32)
            nc.vector.memset(ones[:], 1.0)

            # three accumulator rows in three DIFFERENT psum banks, at
            # partitions 0/32/64 so their matmuls use distinct PE col groups
            v_acc = pacc.tile([1, D], f32, tag="v_acc")
            w_acc = pacc.tile([33, D], f32, tag="w_acc")
            s2_acc = pacc.tile([65, D], f32, tag="s2_acc")

            macc = accp.tile([128, 2 * NT + BLKB * NDC], f32, tag="macc")
            ra = macc[:, 0:NT]
            rb = macc[:, NT : 2 * NT]
            s1c = macc[:, 2 * NT : 2 * NT + BLKB * NDC]

            # ---- first mu/lv slab BEFORE the big x loads so tile-0 compute
            # can start as early as possible; slabs issue on GPSIMD's SWDGE
            # queues to keep trigger descriptor-gen off the Sync sequencer.
            def load_slab(s):
                rows = lv_d[128 * SLAB * s : 128 * SLAB * (s + 1), :]
                lv_sl = slp.tile([128, SLAB * D], f32, tag="lv_sl", name="lv_sl")
                nc.gpsimd.dma_start(
                    lv_sl[:], rows.rearrange("(g p) f -> p g f", p=128)
                )
                rows = mu_d[128 * SLAB * s : 128 * SLAB * (s + 1), :]
                mu_sl = slp.tile([128, SLAB * D], f32, tag="mu_sl", name="mu_sl")
                nc.gpsimd.dma_start(
                    mu_sl[:], rows.rearrange("(g p) f -> p g f", p=128)
                )
                return mu_sl, lv_sl

            slab0 = load_slab(0)

            # ---- load x native: one 2 MiB DMA per b-block ----
            # x_sb[b][p, 1024*dc + hw] = x[b, 128*dc + p, hw]
            x_sb = []
            for b in range(BLKB):
                t_ = xp.tile([128, NDC * HW], f32, tag=f"x_{b}", name=f"x_{b}")
                nc.sync.dma_start(
                    t_[:], x_d[b].rearrange("(dc p) hw -> p dc hw", p=128)
                )
                x_sb.append(t_)

            # ---- main loop over 128-row i-tiles, mu/lv in DMA slabs ----
            mu_sl = lv_sl = None
            for t in range(NT):
                b, j = divmod(t, NT // BLKB)
                s, k = divmod(t, SLAB)
                if k == 0:
                    mu_sl, lv_sl = slab0 if s == 0 else load_slab(s)
                mu_t = mu_sl[:, D * k : D * (k + 1)]
                lv_t = lv_sl[:, D * k : D * (k + 1)]

                # S1 (ACT Copy+accum on native x) spread across mid tiles
                if 4 <= t < 4 + BLKB * NDC:
                    kk = t - 4
                    bb, dcc = divmod(kk, NDC)
                    scr_nat = scrp.tile([128, HW], f32, tag="scr_nat")
                    nc.scalar.activation(
                        scr_nat[:], x_sb[bb][:, HW * dcc : HW * (dcc + 1)], AF.Copy,
                        accum_out=s1c[:, kk : kk + 1],
                    )

                xT = pp.tile([128, D], f32, tag="xT")
                for dc in range(NDC):
                    # 4 chunks share one PSUM bank: one accumulation group
                    nc.tensor.matmul(
                        xT[:, 128 * dc : 128 * (dc + 1)],
                        x_sb[b][:, HW * dc + 128 * j : HW * dc + 128 * (j + 1)],
                        ident[:],
                        is_transpose=True,
                        start=(dc == 0),
                        stop=(dc == NDC - 1),
                    )

                v_t = sp.tile([128, D], f32, tag="v")
                nc.scalar.activation(v_t[:], lv_t, AF.Exp, scale=-1.0)
                p_t = sp.tile([128, D], f32, tag="p")
                nc.scalar.activation(p_t[:], xT[:], AF.Square)

                w_t = sp.tile([128, D], f32, tag="w")
                nc.vector.tensor_tensor(w_t[:], v_t[:], mu_t, OP.mult)

                scr_a = scrp.tile([128, D], f32, tag="scr_a")
                nc.vector.scalar_tensor_tensor(
                    out=scr_a[:], in0=p_t[:], scalar=1.0, in1=v_t[:],
                    op0=OP.mult, op1=OP.mult,
                    accum_out=ra[:, t : t + 1],
                )
                scr_b = scrp.tile([128, D], f32, tag="scr_b")
                nc.vector.scalar_tensor_tensor(
                    out=scr_b[:], in0=w_t[:], scalar=1.0, in1=xT[:],
                    op0=OP.mult, op1=OP.mult,
                    accum_out=rb[:, t : t + 1],
                )

                st, sp_ = (t == 0), (t == NT - 1)
                nc.tensor.matmul(
                    v_acc[0:1, :], ones[:], v_t[:],
                    start=st, stop=sp_, tile_position=(0, 0),
                )
                nc.tensor.matmul(
                    w_acc[32:33, :], ones[:], w_t[:],
                    start=st, stop=sp_, tile_position=(0, 32),
                )
                nc.tensor.matmul(
                    s2_acc[64:65, :], ones[:], p_t[:],
                    start=st, stop=sp_, tile_position=(0, 64),
                )

            # ---- evacuate PSUM rows (lane-locked) -> SBUF -> DRAM ----
            rows_sb = accp.tile([65, D], f32, tag="rows_sb")
            nc.scalar.copy(rows_sb[0:1, :], v_acc[0:1, :])
            nc.scalar.copy(rows_sb[32:33, :], w_acc[32:33, :])
            nc.scalar.copy(rows_sb[64:65, :], s2_acc[64:65, :])

            nc.sync.dma_start(o_vws, rows_sb[0:65:32, :])
            nc.sync.dma_start(o_misc, macc[:])

    nc.compile()
    return nc


def get_program():
    if "nc" not in _prog_cache:
        _prog_cache["nc"] = build_program()
    return _prog_cache["nc"]


def make_in_maps(x, p_mu, p_logvar):
    x = np.ascontiguousarray(np.asarray(x, dtype=np.float32)).reshape(B, D, HW)
    p_mu = np.ascontiguousarray(np.asarray(p_mu, dtype=np.float32))
    p_logvar = np.ascontiguousarray(np.asarray(p_logvar, dtype=np.float32))
    in_maps = []
    for c in range(NCORES):
        in_maps.append(
            {
                "x_s": np.ascontiguousarray(x[BLKB * c : BLKB * (c + 1)]),
                "mu_s": np.ascontiguousarray(p_mu[ROWS * c : ROWS * (c + 1)]),
                "lv_s": np.ascontiguousarray(p_logvar[ROWS * c : ROWS * (c + 1)]),
                "ident": np.eye(128, dtype=np.float32),
            }
        )
    return in_maps


def finish_host(results):
    """Combine per-core partials (float64) into the scalar loss."""
    Vv = np.zeros(D)
    Ww = np.zeros(D)
    S2 = np.zeros(D)
    S1 = np.zeros(D)
    A = 0.0
    Bb = 0.0
    for r in results:
        vws = r["o_vws"].astype(np.float64)
        Vv += vws[0]
        Ww += vws[1]
        S2 += vws[2]
        misc = r["o_misc"].astype(np.float64)
        A += float(misc[:, 0:NT].sum())
        Bb += float(misc[:, NT : 2 * NT].sum())
        s1c = misc[:, 2 * NT :]
        for b in range(BLKB):
            for dc in range(NDC):
                S1[128 * dc : 128 * (dc + 1)] += s1c[:, b * NDC + dc]
    m1 = S1 / N
    m2 = S2 / N
    S = A - 2.0 * Bb - float(np.dot(m2, Vv)) + 2.0 * float(np.dot(m1, Ww))
    return np.float32(-0.5 / N * S)


def run_on_device(x, p_mu, p_logvar, trace=False, **kw):
    from concourse import bass_utils

    nc = get_program()
    in_maps = make_in_maps(x, p_mu, p_logvar)
    return bass_utils.run_bass_kernel_spmd(
        nc, in_maps, list(range(NCORES)), trace=trace, **kw
    )


def kernel(x, p_mu, p_logvar):
    res = run_on_device(x, p_mu, p_logvar)
    return finish_host(res.results)


# revision 27
# speedup vs baseline: 1.0674x; 1.0674x over previous
"""CLUB loss kernel for Trainium2, 8-core data-parallel SPMD.

Math: with flat_x (N,D) [from x (B,D,H,W) -> (B*H*W, D)], v = exp(-p_logvar),
  loss = mean_i[ -0.5*sum_d ((x-mu)^2 - (m2 - 2*mu*m1 + mu^2)) * v ]
       = (-0.5/N) * [ A - 2B - dot(m2, V) + 2*dot(m1, W) ]
where
  A  = sum_{i,d} x^2 v          B  = sum_{i,d} x mu v
  V_d = sum_i v                 W_d = sum_i mu v
  m1 = S1/N, m2 = S2/N,  S1_d = sum_i x,  S2_d = sum_i x^2
All terms are per-core-local partial sums; the tiny (~KB) cross-core
reduction and final dot products happen on host in float64. No collectives.

Layout: everything in d-major (partition = d) so that every reduction above
is a sum over the FREE axis and rides for free on `accum_out` of ops we run
anyway. x streams in natively d-major; mu and logvar are transposed on the
PE (128x128 identity-matmul blocks) into PSUM, and the ACT/DVE consumers
read straight from PSUM, fusing evacuation with compute:
  ACT: v = exp(-lvT)   [PSUM->SBUF]  + accum -> V
  ACT: p = square(x)                 + accum -> S2
  ACT: copy(x)                       + accum -> S1
  DVE: w = muT * v     [PSUM->SBUF]  + accum -> W
  DVE: a = p * v                     + accum -> A-partials
  DVE: b = x * w                     + accum -> B-partials
PE does ONLY the 128 block transposes. No reduction matmuls at all.

Processing unit = (b-block, d-chunk): (128 d) x (1024 i) tiles.
"""

import sys

import numpy as np

for _p in ("/opt/trn_rl_repo",):
    if _p not in sys.path:
        sys.path.append(_p)

B, D, H, W = 16, 512, 32, 32
HW = H * W
N = B * HW
NCORES = 8
BLKB = B // NCORES          # b-blocks per core (2)
ROWS = N // NCORES          # rows per core (2048)
NT = ROWS // 128            # 128-row i-tiles per core (16)
NDC = D // 128              # d chunks (4)
SLAB = 4                    # i-tiles per mu/lv DMA slab
NU = BLKB * NDC             # processing units per core (8)

_prog_cache = {}


def build_program():
    import concourse.bacc as bacc
    import concourse.tile as tile
    from concourse import mybir

    f32 = mybir.dt.float32
    AF = mybir.ActivationFunctionType
    OP = mybir.AluOpType

    nc = bacc.Bacc(
        "TRN2",
        target_bir_lowering=False,
        debug=False,
        enable_asserts=False,
        num_devices=NCORES,
    )

    x_d = nc.dram_tensor("x_s", (BLKB, D, HW), f32, kind="ExternalInput").ap()
    mu_d = nc.dram_tensor("mu_s", (ROWS, D), f32, kind="ExternalInput").ap()
    lv_d = nc.dram_tensor("lv_s", (ROWS, D), f32, kind="ExternalInput").ap()
    id_d = nc.dram_tensor("ident", (128, 128), f32, kind="ExternalInput").ap()

    # o_misc cols, per unit u = b*NDC+dc: 6 quantities x NU units
    # col u*6+q: q=0 V, 1 W, 2 S1, 3 S2, 4 A, 5 B   (partition p -> d=128*dc+p)
    o_misc = nc.dram_tensor("o_misc", (128, 6 * NU), f32, kind="ExternalOutput").ap()

    with tile.TileContext(nc) as tc:
        with (
            tc.tile_pool(name="const", bufs=1) as constp,
            tc.tile_pool(name="xnat", bufs=1) as xp,
            tc.tile_pool(name="slab", bufs=3) as slp,
            tc.tile_pool(name="stream", bufs=2) as sp,
            tc.tile_pool(name="accum", bufs=1) as accp,
            tc.tile_pool(name="psum", bufs=2, space="PSUM") as pp,
        ):
            ident = constp.tile([128, 128], f32)
            nc.sync.dma_start(ident[:], id_d[:])

            macc = accp.tile([128, 6 * NU], f32, tag="macc")

            def load_slab(s):
                rows = lv_d[128 * SLAB * s : 128 * SLAB * (s + 1), :]
                lv_sl = slp.tile([128, SLAB * D], f32, tag="lv_sl", name="lv_sl")
                nc.gpsimd.dma_start(
                    lv_sl[:], rows.rearrange("(g p) f -> p g f", p=128)
                )
                rows = mu_d[128 * SLAB * s : 128 * SLAB * (s + 1), :]
                mu_sl = slp.tile([128, SLAB * D], f32, tag="mu_sl", name="mu_sl")
                nc.gpsimd.dma_start(
                    mu_sl[:], rows.rearrange("(g p) f -> p g f", p=128)
                )
                return mu_sl, lv_sl

            # slabs 0..1 cover b-block 0, slabs 2..3 cover b-block 1.
            slabs = {0: load_slab(0)}

            x_sb = []
            for b in range(BLKB):
                t_ = xp.tile([128, NDC * HW], f32, tag=f"x_{b}", name=f"x_{b}")
                nc.sync.dma_start(
                    t_[:], x_d[b].rearrange("(dc p) hw -> p dc hw", p=128)
                )
                x_sb.append(t_)

            slabs[1] = load_slab(1)

            for u in range(NU):
                b, dc = divmod(u, NDC)
                if b > 0 and dc == 0:
                    slabs[2 * b] = load_slab(2 * b)
                    slabs[2 * b + 1] = load_slab(2 * b + 1)

                # transpose this unit's lv and mu blocks: (1024 i x 128 d)
                # -> PSUM (128 d x 1024 i), 8 blocks each, 4 per psum bank
                lvT = pp.tile([128, HW], f32, tag="lvT", name="lvT")
                muT = pp.tile([128, HW], f32, tag="muT", name="muT")
                for blk in range(8):
                    t_i = 8 * b + blk          # global i-tile index
                    sl = slabs[t_i // SLAB]
                    col = D * (t_i % SLAB) + 128 * dc
                    for dst, src in ((lvT, sl[1]), (muT, sl[0])):
                        nc.tensor.matmul(
                            dst[:, 128 * blk : 128 * (blk + 1)],
                            src[:, col : col + 128],
                            ident[:],
                            is_transpose=True,
                            start=(blk % 4 == 0),
                            stop=(blk % 4 == 3),
                        )

                xs = x_sb[b][:, HW * dc : HW * (dc + 1)]

                v_u = sp.tile([128, HW], f32, tag="v", name="v_u")
                nc.scalar.activation(
                    v_u[:], lvT[:], AF.Exp, scale=-1.0,
                    accum_out=macc[:, 6 * u : 6 * u + 1],
                )
                p_u = sp.tile([128, HW], f32, tag="p", name="p_u")
                nc.scalar.activation(
                    p_u[:], xs, AF.Square,
                    accum_out=macc[:, 6 * u + 3 : 6 * u + 4],
                )
                s1scr = sp.tile([128, HW], f32, tag="s1scr", name="s1scr")
                nc.scalar.activation(
                    s1scr[:], xs, AF.Copy,
                    accum_out=macc[:, 6 * u + 2 : 6 * u + 3],
                )

                w_u = sp.tile([128, HW], f32, tag="w", name="w_u")
                nc.vector.scalar_tensor_tensor(
                    out=w_u[:], in0=muT[:], scalar=1.0, in1=v_u[:],
                    op0=OP.mult, op1=OP.mult,
                    accum_out=macc[:, 6 * u + 1 : 6 * u + 2],
                )
                a_u = sp.tile([128, HW], f32, tag="a", name="a_u")
                nc.vector.scalar_tensor_tensor(
                    out=a_u[:], in0=p_u[:], scalar=1.0, in1=v_u[:],
                    op0=OP.mult, op1=OP.mult,
                    accum_out=macc[:, 6 * u + 4 : 6 * u + 5],
                )
                b_u = sp.tile([128, HW], f32, tag="b", name="b_u")
                nc.vector.scalar_tensor_tensor(
                    out=b_u[:], in0=w_u[:], scalar=1.0, in1=xs,
                    op0=OP.mult, op1=OP.mult,
                    accum_out=macc[:, 6 * u + 5 : 6 * u + 6],
                )

            nc.sync.dma_start(o_misc, macc[:])

    nc.compile()
    return nc


def get_program():
    if "nc" not in _prog_cache:
        _prog_cache["nc"] = build_program()
    return _prog_cache["nc"]


def make_in_maps(x, p_mu, p_logvar):
    x = np.ascontiguousarray(np.asarray(x, dtype=np.float32)).reshape(B, D, HW)
    p_mu = np.ascontiguousarray(np.asarray(p_mu, dtype=np.float32))
    p_logvar = np.ascontiguousarray(np.asarray(p_logvar, dtype=np.float32))
    in_maps = []
    for c in range(NCORES):
        in_maps.append(
            {
                "x_s": np.ascontiguousarray(x[BLKB * c : BLKB * (c + 1)]),
                "mu_s": np.ascontiguousarray(p_mu[ROWS * c : ROWS * (c + 1)]),
                "lv_s": np.ascontiguousarray(p_logvar[ROWS * c : ROWS * (c + 1)]),
                "ident": np.eye(128, dtype=np.float32),
            }
        )
    return in_maps


def finish_host(results):
    """Combine per-core partials (float64) into the scalar loss."""
    Vv = np.zeros(D)
    Ww = np.zeros(D)
    S2 = np.zeros(D)
    S1 = np.zeros(D)
    A = 0.0
    Bb = 0.0
    for r in results:
        misc = r["o_misc"].astype(np.float64)
        for u in range(NU):
            b, dc = divmod(u, NDC)
            dsl = slice(128 * dc, 128 * (dc + 1))
            Vv[dsl] += misc[:, 6 * u]
            Ww[dsl] += misc[:, 6 * u + 1]
            S1[dsl] += misc[:, 6 * u + 2]
            S2[dsl] += misc[:, 6 * u + 3]
            A += float(misc[:, 6 * u + 4].sum())
            Bb += float(misc[:, 6 * u + 5].sum())
    m1 = S1 / N
    m2 = S2 / N
    S = A - 2.0 * Bb - float(np.dot(m2, Vv)) + 2.0 * float(np.dot(m1, Ww))
    return np.float32(-0.5 / N * S)


def run_on_device(x, p_mu, p_logvar, trace=False, **kw):
    from concourse import bass_utils

    nc = get_program()
    in_maps = make_in_maps(x, p_mu, p_logvar)
    return bass_utils.run_bass_kernel_spmd(
        nc, in_maps, list(range(NCORES)), trace=trace, **kw
    )


def kernel(x, p_mu, p_logvar):
    res = run_on_device(x, p_mu, p_logvar)
    return finish_host(res.results)
